# revision 1
# baseline (speedup 1.0000x reference)
"""Trainium2 Bass kernel for nn_CAM_41377714929724 (CAM cross-attention module).

  a1  = f1 @ W                      [B,S,D]
  cc  = a1 @ f2^T                   [B,S,S]
  aatt = softmax(cc, axis=s)        (over rows)
  vatt = softmax(cc, axis=t).T      (over cols, transposed)
  out1 = (f1 @ aatt).swap(1,2)      [B,S,S]
  out2 = (f2 @ vatt).swap(1,2)      [B,S,S]

Sharding: pure data parallelism, 2 batches per core on 8 cores; W replicated.

Per core/batch dataflow (all matmuls fp32r = full PE rate, fp32 PSUM accum):
  a1T[e,s] = sum_d W[d,e] f1T[d,s]          (lhsT=W,    rhs=f1T)
  cc [s,t] = sum_e a1T[e,s] f2T[e,t]        (lhsT=a1T,  rhs=f2T)
  vmax[s]  = max_t cc   (DVE free-dim reduce -> [128,8] stat tile
                         -> PE-transpose -> -vmax row, no DRAM bounce)
  ccT[t,s] = sum_e f2T[e,t] a1T[e,s] - vmax[s]
             (K=1 ones x (-vmax row) matmul appended to the accumulation;
              PSUM drain IS the exp -> e2T[t,s] in one ACT op)
  amax[t]  = max_s cc   (DVE max-combine of 8 tiles + 1 gpsimd partition allreduce)
  e1 [u,t] = exp(cc - amax[t])  in place    (DVE sub + ACT exp)
  asum[x]  = sum_u e1[u,x]  (8x N=1 matmul vs ones column -> [128,1] PSUM,
                             per-partition x directly; 1/asum via DVE recip)
  vsum[s]  = sum_u e2T[u,s] (same)
  out1[x,s] = (sum_u e1[u,x] f1T[u,s]) * (1/asum[x])  (scale fused in PSUM drain)
  out2[s,t] = (sum_u e2T[u,s] f2T[u,t]) * (1/vsum[s])

Column-halved stats keep the PE dense: ret matmuls of one half start while the
other half's stats are in flight.
"""

import numpy as np
from contextlib import ExitStack

import concourse.bass as bass
import concourse.tile as tile
from concourse import bacc, mybir, bass_isa
from concourse.bass_utils import run_bass_kernel_spmd

f32 = mybir.dt.float32
f32r = mybir.dt.float32r

P = 128
N = 1024
NT = N // P          # 8 tiles per matrix dim
NB = 2               # batches per core
NCORES = 8
HALF = 512           # matmul moving free dim / psum bank
Exp = mybir.ActivationFunctionType.Exp
Copy = mybir.ActivationFunctionType.Copy


def _build():
    nc = bacc.Bacc("TRN2", target_bir_lowering=False, debug=False, num_devices=NCORES)

    f1t_d = nc.dram_tensor("f1t", [NB, N, N], f32r, kind="ExternalInput").ap()
    f2t_d = nc.dram_tensor("f2t", [NB, N, N], f32r, kind="ExternalInput").ap()
    w_d = nc.dram_tensor("w", [N, N], f32r, kind="ExternalInput").ap()
    id_d = nc.dram_tensor("ident", [P, P], f32r, kind="ExternalInput").ap()
    o1_d = nc.dram_tensor("o1", [NB, N, N], f32, kind="ExternalOutput").ap()
    o2_d = nc.dram_tensor("o2", [NB, N, N], f32, kind="ExternalOutput").ap()

    with tile.TileContext(nc) as tc, ExitStack() as ctx:
        wp = ctx.enter_context(tc.tile_pool(name="wp", bufs=1))
        f1p = ctx.enter_context(tc.tile_pool(name="f1p", bufs=1))
        f2p = ctx.enter_context(tc.tile_pool(name="f2p", bufs=1))
        a1p = ctx.enter_context(tc.tile_pool(name="a1p", bufs=1))
        ccp = ctx.enter_context(tc.tile_pool(name="ccp", bufs=1))
        cctp = ctx.enter_context(tc.tile_pool(name="cctp", bufs=1))
        statp = ctx.enter_context(tc.tile_pool(name="statp", bufs=1))
        smallp = ctx.enter_context(tc.tile_pool(name="smallp", bufs=1))
        oretp = ctx.enter_context(tc.tile_pool(name="oretp", bufs=4))
        psp = ctx.enter_context(tc.tile_pool(name="psp", bufs=8, space="PSUM"))
        dscrp = ctx.enter_context(tc.tile_pool(name="dscrp", bufs=2, space="DRAM"))

        # constants: fp32r ones (memset can't write f32r), fp32 identity
        ones_f32r_ = smallp.tile([1, P], f32, name="ones_f32r_", tag="ones_f32r_")
        nc.vector.memset(ones_f32r_[:], 1.0)
        ones_k1 = smallp.tile([1, P], f32r, name="ones_k1", tag="ones_k1")
        nc.scalar.copy(ones_k1[:], ones_f32r_[:])
        ones_f32c_ = smallp.tile([P, 1], f32, name="ones_f32c_", tag="ones_f32c_")
        nc.vector.memset(ones_f32c_[:], 1.0)
        ones_col = smallp.tile([P, 1], f32r, name="ones_col", tag="ones_col")
        nc.scalar.copy(ones_col[:], ones_f32c_[:])
        ident = smallp.tile([P, P], f32r, name="ident", tag="ident")
        nc.sync.dma_start(ident[:], id_d[:, :])

        # W is shared by both batches: load once
        ws = []
        f1s_by_b = {}
        for k in range(NT):
            wk = wp.tile([P, N], f32r, name=f"w{k}", tag=f"w{k}")
            nc.sync.dma_start(wk[:], w_d[k * P:(k + 1) * P, :])
            ws.append(wk)
            f1k = f1p.tile([P, N], f32r, name=f"f1_0_{k}", tag=f"f1{k}")
            nc.sync.dma_start(f1k[:], f1t_d[0, k * P:(k + 1) * P, :])
            f1s_by_b.setdefault(0, []).append(f1k)

        for b in range(NB):
            # ---- loads -------------------------------------------------
            if b == 0:
                f1s = f1s_by_b[0]
            else:
                f1s = []
                for k in range(NT):
                    f1k = f1p.tile([P, N], f32r, name=f"f1_{b}_{k}", tag=f"f1{k}")
                    nc.sync.dma_start(f1k[:], f1t_d[b, k * P:(k + 1) * P, :])
                    f1s.append(f1k)
            f2s = []
            for k in range(NT):
                f2k = f2p.tile([P, N], f32r, name=f"f2_{b}_{k}", tag=f"f2{k}")
                nc.sync.dma_start(f2k[:], f2t_d[b, k * P:(k + 1) * P, :])
                f2s.append(f2k)

            def mmgroup(lhs_tiles, rhs_tiles, m, n, drain, tagpfx, extra=None):
                ps = psp.tile([P, HALF], f32, name=f"ps_{tagpfx}", tag="ps")
                for k in range(NT):
                    nc.tensor.matmul(
                        ps[:],
                        lhs_tiles[k][:, m * P:(m + 1) * P],
                        rhs_tiles[k][:, n * HALF:(n + 1) * HALF],
                        start=(k == 0),
                        stop=(k == NT - 1 and extra is None),
                    )
                if extra is not None:
                    extra(ps)
                drain(m, n, ps)

            # ---- a1T[e,s] ----------------------------------------------
            a1s = [a1p.tile([P, N], f32r, name=f"a1_{b}_{m}", tag=f"a1{m}")
                   for m in range(NT)]
            for m in range(NT):
                for n in range(2):
                    mmgroup(ws, f1s, m, n,
                            lambda m_, n_, ps: nc.vector.tensor_copy(
                                a1s[m_][:, n_ * HALF:(n_ + 1) * HALF], ps[:]),
                            "a1")

            # ---- cc[s,t] + vmax stat tile -------------------------------
            ccs = [ccp.tile([P, N], f32r, name=f"cc_{b}_{m}", tag=f"cc{m}")
                   for m in range(NT)]
            nvmax_pp = [smallp.tile([P, 1], f32r, name=f"nvmax_{b}_{m}",
                                    tag=f"vmaxpp{m}") for m in range(NT)]
            scr_v = dscrp.tile([1, N], f32r, name=f"scr_v{b}", tag="scr_v")
            nvrow = statp.tile([1, N], f32r, name=f"nvrow{b}", tag="nvrow")

            def cc_drain(m, n, ps):
                nc.vector.tensor_copy(ccs[m][:, n * HALF:(n + 1) * HALF], ps[:])

            def cc_quad(qrange):
                for m in qrange:
                    for n in range(2):
                        mmgroup(a1s, f2s, m, n, cc_drain, "cc")
                    nc.vector.tensor_reduce(
                        out=nvmax_pp[m][:], in_=ccs[m][:].bitcast(f32),
                        axis=mybir.AxisListType.X, op=mybir.AluOpType.max, negate=True)
                    nc.sync.dma_start(
                        scr_v[0:1, m * P:(m + 1) * P].rearrange(
                            "one (p x) -> (one p) x", p=P),
                        nvmax_pp[m][:])

            # ccT[m-tile][:, q-block of half n] = (cc[4n+q][:, m*128..])^T
            # (PE transpose groups with the K=1 -vmax bias matmul; exp drain -> e2T)
            ccts = [cctp.tile([P, N], f32r, name=f"cct_{b}_{m}", tag=f"cct{m}")
                    for m in range(NT)]

            def cct_transpose_half(n):
                nc.sync.dma_start(nvrow[0:1, n * HALF:(n + 1) * HALF],
                                  scr_v[0:1, n * HALF:(n + 1) * HALF])
                for m in range(NT):
                    ps = psp.tile([P, HALF], f32r, name="ps_t", tag="ps")
                    for q in range(4):
                        nc.tensor.matmul(
                            ps[:, q * P:(q + 1) * P],
                            ccs[4 * n + q][:, m * P:(m + 1) * P], ident[:],
                            is_transpose=True, start=(q == 0), stop=False)
                    nc.tensor.matmul(
                        ps[:].bitcast(f32), ones_k1[:],
                        nvrow[0:1, n * HALF:(n + 1) * HALF],
                        start=False, stop=True)
                    nc.scalar.activation(ccts[m][:, n * HALF:(n + 1) * HALF],
                                         ps[:].bitcast(f32), Exp)

            # ---- column sums via f32r ones-row matmuls + DRAM bounce -----
            rsa = smallp.tile([P, NT], f32, name=f"rsa{b}", tag="rsa")
            rsv = smallp.tile([P, NT], f32, name=f"rsv{b}", tag="rsv")
            scr_s = dscrp.tile([1, 4 * N], f32, name=f"scr_s{b}", tag="scr_s")

            def colsum_mm(tiles, h, col):
                sps = psp.tile([1, HALF], f32, name="sps", tag="ps")
                for k in range(NT):
                    nc.tensor.matmul(
                        sps[:], ones_col[:], tiles[k][:, h * HALF:(h + 1) * HALF],
                        start=(k == 0), stop=(k == NT - 1))
                # hop through SBUF (DMA cannot read PSUM); reuse a dead a1 slot
                srow = a1p.tile([1, HALF], f32, name="sumrow", tag="a10")
                nc.vector.tensor_copy(srow[:], sps[:])
                nc.sync.dma_start(scr_s[0:1, col * HALF:(col + 1) * HALF], srow[:])

            def vsum_half(h):
                colsum_mm(ccts, h, 2 + h)
                rd = scr_s[0:1, (2 + h) * HALF:(3 + h) * HALF].rearrange(
                    "one (m p) -> (one p) m", p=P)
                nc.sync.dma_start(rsv[:, 4 * h:4 * h + 4], rd)
                nc.vector.reciprocal(rsv[:, 4 * h:4 * h + 4], rsv[:, 4 * h:4 * h + 4])

            def ret_drain(out_d, rs, dve=False):
                def d(m, n, ps):
                    ot = oretp.tile([P, HALF], f32, name="oret", tag="oret")
                    if dve:
                        nc.vector.tensor_scalar_mul(ot[:], ps[:], rs[:, m:m + 1])
                    else:
                        nc.scalar.activation(ot[:], ps[:], Copy,
                                             bias=0.0, scale=rs[:, m:m + 1])
                    nc.sync.dma_start(
                        out_d[b, m * P:(m + 1) * P, n * HALF:(n + 1) * HALF], ot[:])
                return d

            # transposes as contiguous blocks (mode switches are expensive),
            # with ret2's first half sandwiched between them: it only needs
            # half-0 e2T drains and keeps the PE warm while ACT catches up
            cc_quad(range(0, NT))
            cct_transpose_half(0)
            vsum_half(0)
            for m in range(0, 4):
                for n in range(2):
                    mmgroup(ccts, f2s, m, n, ret_drain(o2_d, rsv, dve=True), "r2a")
            cct_transpose_half(1)
            vsum_half(1)

            # ---- amax + e1 = exp(cc - amax), per column half -------------
            for h in range(2):
                sl = slice(h * HALF, (h + 1) * HALF)
                amaxt = a1p.tile([P, HALF], f32, name=f"amaxt{b}{h}", tag="a11")
                nc.vector.tensor_copy(amaxt[:], ccs[0][:, sl].bitcast(f32))
                for m in range(1, NT):
                    nc.vector.tensor_tensor(
                        out=amaxt[:], in0=amaxt[:], in1=ccs[m][:, sl].bitcast(f32),
                        op=mybir.AluOpType.max)
                amaxB = a1p.tile([P, HALF], f32, name=f"amaxB{b}{h}", tag="a12")
                nc.gpsimd.partition_all_reduce(
                    amaxB[:], amaxt[:], channels=P, reduce_op=bass_isa.ReduceOp.max)
                for m in range(NT):
                    nc.vector.tensor_tensor(
                        out=ccs[m][:, sl], in0=ccs[m][:, sl].bitcast(f32),
                        in1=amaxB[:], op=mybir.AluOpType.subtract)
                    nc.scalar.activation(ccs[m][:, sl], ccs[m][:, sl].bitcast(f32), Exp)

            for h in range(2):
                colsum_mm(ccs, h, h)          # asum halves at scr_s cols 0,1
            for h in range(2):
                rd = scr_s[0:1, h * HALF:(h + 1) * HALF].rearrange(
                    "one (m p) -> (one p) m", p=P)
                nc.sync.dma_start(rsa[:, 4 * h:4 * h + 4], rd)
            nc.vector.reciprocal(rsa[:], rsa[:])

            for m in range(NT):
                for n in range(2):
                    mmgroup(ccs, f1s, m, n, ret_drain(o1_d, rsa), "r1")

            for m in range(4, NT):
                for n in range(2):
                    mmgroup(ccts, f2s, m, n, ret_drain(o2_d, rsv, dve=True), "r2b")

    nc.compile()
    return nc


_NC = None
TRACE = False
LAST = None


def _get_nc():
    global _NC
    if _NC is None:
        _NC = _build()
    return _NC


def kernel(f1_norm, f2_norm, corr_weights):
    f1_norm = np.ascontiguousarray(f1_norm, dtype=np.float32)
    f2_norm = np.ascontiguousarray(f2_norm, dtype=np.float32)
    w = np.ascontiguousarray(corr_weights, dtype=np.float32)
    B = f1_norm.shape[0]
    assert B == NB * NCORES

    # host-side feature-major transposes: f1t[b] = f1[b].T
    f1t = np.ascontiguousarray(np.swapaxes(f1_norm, 1, 2))
    f2t = np.ascontiguousarray(np.swapaxes(f2_norm, 1, 2))
    ident = np.eye(P, dtype=np.float32)

    nc = _get_nc()
    in_maps = [
        {"f1t": f1t[c * NB:(c + 1) * NB], "f2t": f2t[c * NB:(c + 1) * NB],
         "w": w, "ident": ident}
        for c in range(NCORES)
    ]
    res = run_bass_kernel_spmd(nc, in_maps, core_ids=list(range(NCORES)), trace=TRACE)
    global LAST
    LAST = res
    out1 = np.concatenate([res.results[c]["o1"] for c in range(NCORES)], axis=0)
    out2 = np.concatenate([res.results[c]["o2"] for c in range(NCORES)], axis=0)
    return out1, out2



# revision 8
# speedup vs baseline: 1.1185x; 1.1185x over previous
"""Trainium2 Bass kernel for nn_CAM_41377714929724 (CAM cross-attention module).

  a1  = f1 @ W                      [B,S,D]
  cc  = a1 @ f2^T                   [B,S,S]
  aatt = softmax(cc, axis=s)        (over rows -> column-normalized)
  vatt = softmax(cc, axis=t).T      (over cols, transposed)
  out1 = (f1 @ aatt).swap(1,2)      [B,S,S]
  out2 = (f2 @ vatt).swap(1,2)      [B,S,S]

Sharding: pure data parallelism, 2 batches per core on 8 cores; W replicated.

v2 design: PE runs ONLY the four 1024^3 GEMMs per batch; everything else is
pushed to DVE/ACT/gpsimd/DMA so the tensor engine never idles and never runs
transpose-mode (which resets the HAM clock gate).

Per batch (all tiles consolidated as [128, 8*1024] with k-tile = column block):
  G1 (f32r): a1T[e,s] = sum_d W[d,e] f1T[d,s]      drains -> a1 (DVE)
  G2 (f32r): cc[s,t]  = sum_e a1T[e,s] f2T[e,t]    drains -> cc roll (ACT)
     per m-tile: spill cc->DRAM (DMA), vmax row-max (DVE), e2 = exp(cc-vmax)
     as bf16 (ACT, per-partition bias), vsum row-sum (DVE), running column
     max for amax (DVE), e2T via DMA xbar transpose (2 calls, 3D out AP)
  amax = gpsimd partition all-reduce; e1 = exp(cc_reload - amax) bf16
     (DVE sub + ACT exp); asum = DVE tile-add + gpsimd all-reduce + DRAM
     bounce -> rsa per-partition scale
  G4 (bf16): out2[s,u] = (sum_t e2T[t,s] f2T[t,u]) / vsum[s]   (DVE drain)
  G3 (bf16): out1[t,s'] = (sum_i e1[i,t] f1T[i,s']) / asum[t]  (ACT drain)

Cross-batch PE order hides all softmax latency:
  b0.G1 b0.G2 | b1.G1 | b0.G4 b0.G3[0:6] | b1.G2 | b0.G3[6:8] | b1.G4 b1.G3
"""

import numpy as np
import ml_dtypes
from contextlib import ExitStack

import concourse.bass as bass
import concourse.tile as tile
from concourse import bacc, mybir, bass_isa
from concourse.bass_utils import run_bass_kernel_spmd

f32 = mybir.dt.float32
f32r = mybir.dt.float32r
bf16 = mybir.dt.bfloat16

P = 128
N = 1024
NT = N // P          # 8 tiles per matrix dim
NB = 2               # batches per core
NCORES = 8
HALF = 512
BIG = NT * N         # 8192 columns in a consolidated tile
Exp = mybir.ActivationFunctionType.Exp
Copy = mybir.ActivationFunctionType.Copy


def _build():
    nc = bacc.Bacc("TRN2", target_bir_lowering=False, debug=False, num_devices=NCORES)

    f1t_d = nc.dram_tensor("f1t", [NB, N, N], f32r, kind="ExternalInput").ap()
    f2t_d = nc.dram_tensor("f2t", [NB, N, N], f32r, kind="ExternalInput").ap()
    f1b_d = nc.dram_tensor("f1b", [NB, N, N], bf16, kind="ExternalInput").ap()
    f2b_d = nc.dram_tensor("f2b", [NB, N, N], bf16, kind="ExternalInput").ap()
    w_d = nc.dram_tensor("w", [N, N], f32r, kind="ExternalInput").ap()
    o1_d = nc.dram_tensor("o1", [NB, N, N], f32, kind="ExternalOutput").ap()
    o2_d = nc.dram_tensor("o2", [NB, N, N], f32, kind="ExternalOutput").ap()

    with tile.TileContext(nc) as tc, ExitStack() as ctx:
        # 4MB-class slots, multi-tenant via shared tags (sequential lifetimes)
        bigp = ctx.enter_context(tc.tile_pool(name="bigp", bufs=1))
        # rolling small pools
        ccp = ctx.enter_context(tc.tile_pool(name="ccp", bufs=2))
        ccrp = ctx.enter_context(tc.tile_pool(name="ccrp", bufs=2))
        e2p = ctx.enter_context(tc.tile_pool(name="e2p", bufs=3))
        ostp = ctx.enter_context(tc.tile_pool(name="ostp", bufs=2))
        ost2p = ctx.enter_context(tc.tile_pool(name="ost2p", bufs=2))
        statp = ctx.enter_context(tc.tile_pool(name="statp", bufs=1))
        stat2p = ctx.enter_context(tc.tile_pool(name="stat2p", bufs=2))
        psp = ctx.enter_context(tc.tile_pool(name="psp", bufs=1, space="PSUM"))
        dscrp = ctx.enter_context(tc.tile_pool(name="dscrp", bufs=2, space="DRAM"))

        # ---- static big slots --------------------------------------------
        # "w":  W (b0.G1,b1.G1) -> post_b1 = [e1_b1 | f1b_b1] bf16
        # "f1": f1_b0 -> f1_b1 -> f2_b1 (f32r)
        # "f2": f2_b0 -> post_b0 = [e1_b0 | f1b_b0] bf16
        # "a1": a1_b0 -> a1_b1 (f32r)
        wt = bigp.tile([P, BIG], f32r, name="w", tag="w")
        for kk in range(NT):
            nc.sync.dma_start(wt[:, kk * N:(kk + 1) * N], w_d[kk * P:(kk + 1) * P, :])

        def load_big_simple(dst, src_d, b, dt):
            # one DMA per k-tile: dst cols [k*N,(k+1)*N) <- src_d[b, k*128:(k+1)*128, :]
            for kk in range(NT):
                nc.sync.dma_start(dst[:, kk * N:(kk + 1) * N],
                                  src_d[b, kk * P:(kk + 1) * P, :])

        per_b = []
        for b in range(NB):
            d = {}
            d["ccsp"] = dscrp.tile([P, BIG], f32, name=f"ccsp{b}", tag="ccsp")
            d["scr"] = dscrp.tile([1, N], f32, name=f"scr{b}", tag="scr")
            d["nvmax"] = stat2p.tile([P, NT], f32, name=f"nvmax{b}", tag="nvmax")
            d["vs"] = stat2p.tile([P, NT], f32, name=f"vs{b}", tag="vs")
            d["rv"] = stat2p.tile([P, NT], f32, name=f"rv{b}", tag="rv")
            d["rsa"] = stat2p.tile([P, NT], f32, name=f"rsa{b}", tag="rsa")
            per_b.append(d)

        # GEMM helper: k-inner over both n halves, weights shared per (m,k)
        def gemm(lhsT_sl, rhs_sl, m, drain, pstag):
            """lhsT_sl(k,m)->AP [128,128]; rhs_sl(k,n)->AP [128,512]."""
            ps0 = psp.tile([P, HALF], f32, name=f"ps_{pstag}_{m}_0",
                           tag=f"ps{(m % 4) * 2}")
            ps1 = psp.tile([P, HALF], f32, name=f"ps_{pstag}_{m}_1",
                           tag=f"ps{(m % 4) * 2 + 1}")
            for k in range(NT):
                nc.tensor.matmul(ps0[:], lhsT_sl(k, m), rhs_sl(k, 0),
                                 start=(k == 0), stop=(k == NT - 1))
                nc.tensor.matmul(ps1[:], lhsT_sl(k, m), rhs_sl(k, 1),
                                 start=(k == 0), stop=(k == NT - 1))
            drain(m, 0, ps0)
            drain(m, 1, ps1)

        def sl_big(t):
            return lambda k, m: t[:, k * N + m * P: k * N + (m + 1) * P]

        def sl_rhs(t):
            return lambda k, n: t[:, k * N + n * HALF: k * N + (n + 1) * HALF]

        # ---------------- phase builders ----------------------------------
        state = {}

        def g1(b, f1_t, a1_t):
            def drain(m, n, ps):
                nc.vector.tensor_copy(
                    a1_t[:, m * N + n * HALF: m * N + (n + 1) * HALF], ps[:])
            for m in range(NT):
                gemm(sl_big(wt), sl_rhs(f1_t), m, drain, f"a1_{b}")

        def g2(b, f2_t, a1_t, e2t_t):
            d = per_b[b]
            amaxacc = statp.tile([P, N], f32, name=f"amaxacc{b}", tag="amaxacc")
            ccs = []
            e2s = []

            def drain(m, n, ps):
                nc.scalar.copy(
                    ccs[m][:, n * HALF:(n + 1) * HALF], ps[:])

            def vsum(m):
                nc.vector.tensor_reduce(
                    out=d["vs"][:, m:m + 1], in_=e2s[m][:],
                    axis=mybir.AxisListType.X, op=mybir.AluOpType.add)

            for m in range(NT):
                cct = ccp.tile([P, N], f32, name=f"cc_{b}_{m}", tag="cc")
                ccs.append(cct)
                gemm(sl_big(a1_t), sl_rhs(f2_t), m, drain, f"cc_{b}")
                # spill for the e1 path
                nc.sync.dma_start(d["ccsp"][:, m * N:(m + 1) * N], cct[:])
                # row stats + exp -> e2 bf16
                nc.vector.tensor_reduce(
                    out=d["nvmax"][:, m:m + 1], in_=cct[:],
                    axis=mybir.AxisListType.X, op=mybir.AluOpType.max,
                    negate=True)
                e2t = e2p.tile([P, N], bf16, name=f"e2_{b}_{m}", tag="e2")
                e2s.append(e2t)
                nc.scalar.activation(e2t[:], cct[:], Exp,
                                     bias=d["nvmax"][:, m:m + 1])
                # running column max for amax
                if m == 0:
                    nc.vector.tensor_copy(amaxacc[:], cct[:])
                else:
                    nc.vector.tensor_tensor(
                        out=amaxacc[:], in0=amaxacc[:], in1=cct[:],
                        op=mybir.AluOpType.max)
                # vsum lags one m so the DVE never waits on this m's ACT exp
                if m > 0:
                    vsum(m - 1)
                # e2T via DMA xbar transpose: 2 calls of [128,512]
                # out[p, j, c] = in[c, j*128 + p]
                e2tv = e2t_t[:, :].rearrange("p (j c) -> p j c", c=N)
                for j0 in range(2):
                    nc.sync.dma_start(
                        e2tv[:, 4 * j0:4 * j0 + 4, m * P:(m + 1) * P],
                        e2t[:, j0 * HALF:(j0 + 1) * HALF],
                        transpose=True)
            vsum(NT - 1)
            nc.vector.reciprocal(d["rv"][:], d["vs"][:])
            state[b] = dict(amaxacc=amaxacc)

        def softmax1(b, e1_t):
            """amax all-reduce, e1 = exp(cc - amax) bf16, asum -> rsa."""
            d = per_b[b]
            amaxacc = state[b]["amaxacc"]
            amaxB = statp.tile([P, N], f32, name=f"amaxB{b}", tag="amaxB")
            nc.gpsimd.partition_all_reduce(
                amaxB[:], amaxacc[:], channels=P,
                reduce_op=bass_isa.ReduceOp.max)
            asumacc = statp.tile([P, N], f32, name=f"asumacc{b}", tag="asumacc")
            for m in range(NT):
                ccr = ccrp.tile([P, N], f32, name=f"ccr_{b}_{m}", tag="ccr")
                nc.scalar.dma_start(ccr[:], d["ccsp"][:, m * N:(m + 1) * N])
                nc.vector.tensor_tensor(
                    out=ccr[:], in0=ccr[:], in1=amaxB[:],
                    op=mybir.AluOpType.subtract)
                nc.scalar.activation(e1_t[:, m * N:(m + 1) * N], ccr[:], Exp)
                if m == 0:
                    nc.vector.tensor_copy(asumacc[:],
                                          e1_t[:, m * N:(m + 1) * N])
                else:
                    nc.vector.tensor_tensor(
                        out=asumacc[:], in0=asumacc[:],
                        in1=e1_t[:, m * N:(m + 1) * N],
                        op=mybir.AluOpType.add)
            asumB = statp.tile([P, N], f32, name=f"asumB{b}", tag="amaxB")
            nc.gpsimd.partition_all_reduce(
                asumB[:], asumacc[:], channels=P,
                reduce_op=bass_isa.ReduceOp.add)
            # bounce row -> per-partition column layout
            nc.sync.dma_start(d["scr"][0:1, :], asumB[0:1, :])
            nc.sync.dma_start(
                d["rsa"][:],
                d["scr"][0:1, :].rearrange("one (m p) -> (one p) m", p=P))
            nc.vector.reciprocal(d["rsa"][:], d["rsa"][:])

        def g4(b, e2t_t, f2b_t, ms=range(NT)):
            d = per_b[b]
            for m in ms:
                def drain(m_, n, ps):
                    ost = ostp.tile([P, HALF], f32, name=f"ost4_{b}", tag="ost")
                    nc.vector.tensor_scalar_mul(
                        ost[:], ps[:], d["rv"][:, m_:m_ + 1])
                    nc.sync.dma_start(
                        o2_d[b, m_ * P:(m_ + 1) * P,
                             n * HALF:(n + 1) * HALF], ost[:])
                gemm(sl_big(e2t_t), sl_rhs(f2b_t), m, drain, f"r2_{b}")

        def g3(b, e1_t, f1b_t, ms):
            d = per_b[b]
            for m in ms:
                def drain(m_, n, ps):
                    ost = ost2p.tile([P, HALF], f32, name=f"ost3_{b}", tag="ost2")
                    nc.scalar.activation(
                        ost[:], ps[:], Copy,
                        bias=0.0, scale=d["rsa"][:, m_:m_ + 1])
                    nc.sync.dma_start(
                        o1_d[b, m_ * P:(m_ + 1) * P,
                             n * HALF:(n + 1) * HALF], ost[:])
                gemm(sl_big(e1_t), sl_rhs(f1b_t), m, drain, f"r1_{b}")

        # ---------------- global schedule ---------------------------------
        # slot tenants
        f1_0 = bigp.tile([P, BIG], f32r, name="f1_0", tag="f1")
        f2_0 = bigp.tile([P, BIG], f32r, name="f2_0", tag="f2")
        a1_0 = bigp.tile([P, BIG], f32r, name="a1_0", tag="a1")
        e2t_0 = bigp.tile([P, BIG], bf16, name="e2t_0", tag="e2t")
        f2b_0 = bigp.tile([P, BIG], bf16, name="f2b_0", tag="f2b")

        load_big_simple(f1_0, f1t_d, 0, f32r)
        load_big_simple(f2_0, f2t_d, 0, f32r)
        for kk in range(NT):
            nc.sync.dma_start(f2b_0[:, kk * N:(kk + 1) * N],
                              f2b_d[0, kk * P:(kk + 1) * P, :])

        # P1: b0.G1
        g1(0, f1_0, a1_0)

        # loads for b1.G1 (into "f1" slot, free after b0.G1)
        f1_1 = bigp.tile([P, BIG], f32r, name="f1_1", tag="f1")
        load_big_simple(f1_1, f1t_d, 1, f32r)

        # P2: b0.G2 (+ per-m stats/spill/transpose)
        g2(0, f2_0, a1_0, e2t_0)

        # post_b0 in "f2" slot: [e1_b0 | f1b_b0]
        post_0 = bigp.tile([P, 2 * BIG], bf16, name="post_0", tag="f2")
        e1_0 = post_0[:, 0:BIG]
        f1b_0 = post_0[:, BIG:2 * BIG]
        for kk in range(NT):
            nc.sync.dma_start(f1b_0[:, kk * N:(kk + 1) * N],
                              f1b_d[0, kk * P:(kk + 1) * P, :])

        # P3: b1.G1
        a1_1 = bigp.tile([P, BIG], f32r, name="a1_1", tag="a1")
        g1(1, f1_1, a1_1)

        # b0 softmax-1 path (runs on DVE/ACT/gpsimd during P3)
        softmax1(0, e1_0)

        # loads for b1.G2/G4 (f2_1 into "f1" slot, free after b1.G1)
        f2_1 = bigp.tile([P, BIG], f32r, name="f2_1", tag="f1")
        load_big_simple(f2_1, f2t_d, 1, f32r)
        f2b_1 = bigp.tile([P, BIG], bf16, name="f2b_1", tag="f2b")
        for kk in range(NT):
            nc.sync.dma_start(f2b_1[:, kk * N:(kk + 1) * N],
                              f2b_d[1, kk * P:(kk + 1) * P, :])

        # P4: b0.G4
        g4(0, e2t_0, f2b_0)

        # P5: b0.G3 first 6 m-tiles
        g3(0, e1_0, f1b_0, range(0, 6))

        # P6: b1.G2
        e2t_1 = bigp.tile([P, BIG], bf16, name="e2t_1", tag="e2t")
        g2(1, f2_1, a1_1, e2t_1)

        # post_b1 in "w" slot
        post_1 = bigp.tile([P, 2 * BIG], bf16, name="post_1", tag="w")
        e1_1 = post_1[:, 0:BIG]
        f1b_1 = post_1[:, BIG:2 * BIG]
        for kk in range(NT):
            nc.sync.dma_start(f1b_1[:, kk * N:(kk + 1) * N],
                              f1b_d[1, kk * P:(kk + 1) * P, :])

        softmax1(1, e1_1)

        # P7: b0.G3 tail (PE filler while b1 e2T/rv settle), then b1.G4
        g3(0, e1_0, f1b_0, range(6, NT))
        g4(1, e2t_1, f2b_1)

        # P8: b1.G3
        g3(1, e1_1, f1b_1, range(NT))

    nc.compile()
    return nc


_NC = None
TRACE = False
LAST = None


def _get_nc():
    global _NC
    if _NC is None:
        _NC = _build()
    return _NC


def kernel(f1_norm, f2_norm, corr_weights):
    f1_norm = np.ascontiguousarray(f1_norm, dtype=np.float32)
    f2_norm = np.ascontiguousarray(f2_norm, dtype=np.float32)
    w = np.ascontiguousarray(corr_weights, dtype=np.float32)
    B = f1_norm.shape[0]
    assert B == NB * NCORES

    # host-side feature-major transposes: f1t[b] = f1[b].T
    f1t = np.ascontiguousarray(np.swapaxes(f1_norm, 1, 2))
    f2t = np.ascontiguousarray(np.swapaxes(f2_norm, 1, 2))
    f1b = f1t.astype(ml_dtypes.bfloat16)
    f2b = f2t.astype(ml_dtypes.bfloat16)

    nc = _get_nc()
    in_maps = [
        {"f1t": f1t[c * NB:(c + 1) * NB], "f2t": f2t[c * NB:(c + 1) * NB],
         "f1b": f1b[c * NB:(c + 1) * NB], "f2b": f2b[c * NB:(c + 1) * NB],
         "w": w}
        for c in range(NCORES)
    ]
    res = run_bass_kernel_spmd(nc, in_maps, core_ids=list(range(NCORES)), trace=TRACE)
    global LAST
    LAST = res
    out1 = np.concatenate([res.results[c]["o1"] for c in range(NCORES)], axis=0)
    out2 = np.concatenate([res.results[c]["o2"] for c in range(NCORES)], axis=0)
    return out1, out2
